# revision 59
# baseline (speedup 1.0000x reference)
"""Multi-head attention (B=2, QL=KL=2048, DIM=1024, H=16) on 8 TRN2 NeuronCores.

Sharding: core c handles batch c//4 and heads (c%4)*4 .. (c%4)*4+4 (column-
parallel q/k/v projections, row-parallel out projection). Each core emits a
partial output [QL, DIM]; the host sums the 4 partials per batch and adds the
output bias (the row-parallel all-reduce, done at unshard time).

Per-core kernel (all matmul operands bf16, fp32 PSUM accumulation):
  - activations loaded feature-major (x^T) via host pre-transpose; weights
    host-pre-arranged partition-major so weight DMAs are contiguous.
  - attention runs head-PAIR blocks over 512-query tiles: the two heads of a
    128-partition group issue their K=64 score matmuls back-to-back at PE row
    groups 0 and 64, so they execute concurrently in the PE array (halves
    score time vs sequential K=64 matmuls).
  - score PSUM is triple-buffered (tag a = 3x2 banks; the out-projection
    shares this rotation) so the exp latency hides behind the j-loop.
  - softmax exp is split across engines per a static schedule: scalar ACT
    computes 9/16 [128, 1024] chunks; the vector engine computes 7/16 with a
    1-instruction Schraudolph bf16 exp (~1.9% rms), scheduled mid-to-late in
    each block so vector can drain the rescale chain at block boundaries.
  - PV lhsT is [64 v-dims | 64 ones]: PSUM partitions 64-127 accumulate 64
    broadcast copies of the softmax denominators, so 1/d arrives via one
    cross-base tensor_copy + one reciprocal (no DRAM hops; NOTE reciprocal
    only works at base partition 0, cross-base tensor_tensor is illegal).
  - the rescale multiply runs on gpsimd (vector for the last block, since
    gpsimd queue+drain latency would sit on the critical path there).
  - the out-projection is interleaved into the attention j-loop late in each
    block (the previous ig's rescale chain must land in attnTb first, and
    chunks emitted after a later attnTb write falsely wait on it — the
    dependency tracking is coarse); blocks 6-7 hold back chunks so the PE
    stays busy through the final block's chain; partial outputs leave as
    bf16 and the host reduction sums them in fp32.
"""

import numpy as np
import ml_dtypes

import concourse.bass as bass
import concourse.mybir as mybir
import concourse.tile as tile
from concourse import bacc
from concourse.bass_utils import run_bass_kernel_spmd

BF16 = mybir.dt.bfloat16
F32 = mybir.dt.float32
I32 = mybir.dt.int32
I16 = mybir.dt.int16

B = 2
DIM = 1024
NUM_HEADS = 16
HD = DIM // NUM_HEADS  # 64
SCALE = HD ** -0.5
NCORES = 8
NH = 4          # heads per core
CDIM = NH * HD  # 256, per-core slice of the head dim
P = 128
IT = 512        # projection output tile / attention query block
ECH = DIM // P  # 8 contraction chunks for the projections

# ---------------------------------------------------------------------------
# Custom-DVE exp (accurate 2-inst variant, kept for accuracy tuning):
# inst1 (stock tensor_scalar) i32 = convert(x*A + B) whose bitcast is
# Schraudolph's 2^t*(1+f); inst2 (custom op) extracts w = 1+f via bitwise
# mask/or and multiplies by the deg-2 correction h(f) ~ 2^f/(1+f).
# ---------------------------------------------------------------------------
from concourse import dve_spec
from concourse.dve_spec import Spec, Src0, C0, C1, C2, C3, One, Bin, AluOp, lower
from concourse.dve_ops import (DveOp, DveOpSpec, OPS, CUSTOM_DVE_SPECS,
                               _SUB_OPCODE_FOR_NAME)

EXP_C0 = 0.996535552615256
EXP_C1 = -0.22675587587121068
EXP_C2 = 0.23368355838831362
MASK_F = float(np.int32(0x007FFFFF).view(np.float32))
EXPA_A = float(2**23 * np.log2(np.e) * SCALE)   # i32 Schraudolph (accurate)
EXPA_B = float(127 * 2**23)
EXPF_A = float(2**7 * np.log2(np.e) * SCALE)    # i16 Schraudolph (fast bf16)
EXPF_B = float(127 * 2**7) - 7.366


def _ref_exp_mancor(in0, in1, s0, s1, imm2):
    bits = in0.view(np.int32)
    w = ((bits & 0x007FFFFF) | 0x3F800000).view(np.float32)
    f = w - 1.0
    return in0 * (s1 + f * (imm2 + f * in1))


def _make_op():
    name = "EXP_MANCOR_ANT"
    if name in _SUB_OPCODE_FOR_NAME:
        return next(o for o in OPS if o.name == name)
    m = Bin(AluOp.BITWISE_AND, Src0, C0)
    w = Bin(AluOp.BITWISE_OR, m, One)
    f = w - One
    h = C1 + f * (C2 + f * C3)
    spec = Spec(body=dve_spec._spill_c3_to_src1(h * Src0),
                reference=_ref_exp_mancor)
    opcode = max(_SUB_OPCODE_FOR_NAME.values()) + 1
    _SUB_OPCODE_FOR_NAME[name] = opcode
    shas = {}
    for ver in ("v3", "v4"):
        s = DveOpSpec(name=name, opcode=opcode, uops=lower(spec, ver=ver),
                      rd1_en=True)
        shas[ver] = s.sha(ver)
    op = DveOp(name, spec, subdim=False, uops_sha=shas)
    OPS.append(op)
    CUSTOM_DVE_SPECS[name] = spec
    return op


EXP_MANCOR = _make_op()

# exp-engine assignment: 'S' scalar ACT, 'F' DVE fast (1 inst),
# 'A' DVE accurate (2 inst).  7 DVE / 9 scalar chunks per 16-j block keeps
# scalar+vector each just under the block's PE time; the F chunks sit mid-to-
# late so the vector engine can drain the previous block's rescale chain
# during the scalar-covered early js.
_F_SET = {4, 6, 7, 9, 11, 12, 14}


def exp_mode(blk, j):
    return 'F' if j in _F_SET else 'S'


def build_bass(QL=2048, KL=2048, num_devices=NCORES, dbg=False):
    assert QL % IT == 0 and KL % 256 == 0
    NJC = KL // P    # j chunks (16)
    IG = IT          # 512-query block
    NIG = QL // IG   # 4

    nc = bacc.Bacc("TRN2", target_bir_lowering=False, debug=False,
                   num_devices=num_devices)
    if dbg:
        d_den = nc.dram_tensor("d_den", [24, 2 * IG], F32,
                               kind="ExternalOutput").ap()
        d_att = nc.dram_tensor("d_att", [P, 2, QL], BF16,
                               kind="ExternalOutput").ap()
    qb = nc.dram_tensor("qbT", [DIM, QL], BF16, kind="ExternalInput").ap()
    kb = nc.dram_tensor("kbT", [DIM, KL], BF16, kind="ExternalInput").ap()
    vb = nc.dram_tensor("vbT", [DIM, KL], BF16, kind="ExternalInput").ap()
    # weights arrive host-pre-arranged partition-major so the loads are
    # contiguous 2KB-per-partition DMAs instead of 512B scatters
    wqT = nc.dram_tensor("wqT", [P, ECH, CDIM], BF16, kind="ExternalInput").ap()
    wkT = nc.dram_tensor("wkT", [P, ECH, CDIM], BF16, kind="ExternalInput").ap()
    wvT = nc.dram_tensor("wvT", [P, ECH, CDIM], BF16, kind="ExternalInput").ap()
    woT = nc.dram_tensor("woT", [P, CDIM // P, DIM], BF16,
                         kind="ExternalInput").ap()
    outp = nc.dram_tensor("outp", [QL, DIM], BF16, kind="ExternalOutput").ap()

    with tile.TileContext(nc) as tc:
        with (
            tc.tile_pool(name="wpool", bufs=1) as wpool,
            tc.tile_pool(name="xpool", bufs=12) as xpool,
            tc.tile_pool(name="ptpool", bufs=10) as ptpool,
            tc.tile_pool(name="sxpool", bufs=2) as sxpool,
            tc.tile_pool(name="rpool", bufs=4) as rpool,
            tc.tile_pool(name="stpool", bufs=4) as stpool,
            tc.tile_pool(name="opool", bufs=4) as opool,
            tc.tile_pool(name="psum", bufs=2, space="PSUM") as psum,
        ):
            # ---- persistent SBUF tensors ----
            wq_sb = wpool.tile([P, ECH, CDIM], BF16, tag="wq")
            wk_sb = wpool.tile([P, ECH, CDIM], BF16, tag="wk")
            wv_sb = wpool.tile([P, ECH, CDIM], BF16, tag="wv")
            wo_sb = wpool.tile([P, CDIM // P, DIM], BF16, tag="wo")
            nc.scalar.dma_start(wq_sb[:], wqT)

            qhT = wpool.tile([P, CDIM // P, QL], BF16, tag="qhT")
            khT = wpool.tile([P, CDIM // P, KL], BF16, tag="khT")
            # v lhsT is [64 v-dims | 64 ones]: the ones half makes PSUM
            # partitions 64-127 accumulate 64 broadcast copies of the
            # softmax denominators (matmul time is N-bound, so it's free)
            vh = wpool.tile([P, NJC, NH, P], BF16, tag="vh")
            nc.gpsimd.memset(vh[:, :, :, HD:P], 1.0)
            attnTb = wpool.tile([P, CDIM // P, QL], BF16, tag="attnTb")
            c2t = wpool.tile([P, 1], F32, tag="c2t")
            nc.gpsimd.memset(c2t[:], EXP_C2)

            # psum: tag a = 3x[128,1024] (6 banks, triple-buffers the score
            # tiles so the exp latency hides), tag b = 2x[128,512] (2 banks)
            def acc_tiles():
                a0 = psum.tile([P, 2 * IT], F32, tag="a", bufs=3, name="a0")
                a1 = psum.tile([P, 2 * IT], F32, tag="a", bufs=3, name="a1")
                a2 = psum.tile([P, 2 * IT], F32, tag="a", bufs=3, name="a2")
                b0 = psum.tile([P, IT], F32, tag="b", bufs=2, name="b0")
                b1 = psum.tile([P, IT], F32, tag="b", bufs=2, name="b1")
                return a0, a1, a2, b0, b1

            # ---- phase 1: q/k projections (out: [d'(256) part-major, token]) ----
            def proj_qk(x_dram, w_sb, dst, L, after_e=None):
                # 8 output pairs [128, IT]: pairs 0-3 in a0/a1, 4,5 in b, 6,7 in c
                # copies go on vector only: the scalar engine must stay free
                # to issue the NEXT projection's x-tile DMAs ahead of time
                a0, a1, a2, b0, b1 = acc_tiles()
                slots = [a0[:, 0:IT], a0[:, IT:2 * IT], a1[:, 0:IT],
                         a1[:, IT:2 * IT], a2[:, 0:IT], a2[:, IT:2 * IT],
                         b0[:], b1[:]]
                for e in range(ECH):
                    xT = xpool.tile([P, L], BF16, tag="xT")
                    eng = nc.sync if e % 2 == 0 else nc.scalar
                    eng.dma_start(xT[:], x_dram[e * P:(e + 1) * P, :])
                    if after_e is not None and e in after_e:
                        after_e[e]()
                    for d in range(2):
                        for it in range(L // IT):
                            pair = d * (L // IT) + it
                            nc.tensor.matmul(slots[pair],
                                             lhsT=w_sb[:, e, d * P:(d + 1) * P],
                                             rhs=xT[:, it * IT:(it + 1) * IT],
                                             start=(e == 0), stop=(e == ECH - 1))
                for d in range(2):
                    for it in range(L // IT):
                        pair = d * (L // IT) + it
                        nc.vector.tensor_copy(
                            dst[:, d, it * IT:(it + 1) * IT], slots[pair])

            # wk/wv loads slot between q's x-tile issues so they don't delay
            # the early x tiles on the scalar ring
            proj_qk(qb, wq_sb, qhT, QL, after_e={
                1: lambda: nc.scalar.dma_start(wk_sb[:], wkT),
                3: lambda: nc.scalar.dma_start(wv_sb[:], wvT),
            })
            proj_qk(kb, wk_sb, khT, KL)

            # ---- v projection (out: [j part-major, head dim], ones-last) ----
            HALF = KL // 2
            for jg in range(2):
                njc_h = NJC // 2  # 8 j-chunks in this half
                # one chunk per PSUM bank: start=True clears has_written at
                # bank granularity, so accumulation chains must not share one
                va0, va1, va2, vb0, vb1 = acc_tiles()
                vslots = [va0[:, 0:CDIM], va0[:, IT:IT + CDIM],
                          va1[:, 0:CDIM], va1[:, IT:IT + CDIM],
                          va2[:, 0:CDIM], va2[:, IT:IT + CDIM],
                          vb0[:, 0:CDIM], vb1[:, 0:CDIM]]
                for e in range(ECH):
                    vT = xpool.tile([P, HALF], BF16, tag="xT")
                    eng = nc.sync if e % 2 == 0 else nc.scalar
                    eng.dma_start(
                        vT[:], vb[e * P:(e + 1) * P, jg * HALF:(jg + 1) * HALF])
                    for jc in range(njc_h):
                        nc.tensor.matmul(vslots[jc],
                                         lhsT=vT[:, jc * P:(jc + 1) * P],
                                         rhs=wv_sb[:, e, :],
                                         start=(e == 0), stop=(e == ECH - 1))
                for jc in range(njc_h):
                    j = jg * njc_h + jc
                    nc.vector.tensor_copy(
                        vh[:, j, :, 0:HD],
                        vslots[jc].rearrange("p (h c) -> p h c", h=NH))

            nc.scalar.dma_start(wo_sb[:], woT)

            # ---- phase 2: attention (head-pair x 512 queries per block) ----
            pending_oproj = []
            oproj_ct = [0]

            def emit_oproj_chunk(dma_eng=None, copy_eng=None):
                if not pending_oproj:
                    return
                kw = {}
                if dma_eng is not None:
                    kw["dma_eng"] = dma_eng
                if copy_eng is not None:
                    kw["copy_eng"] = copy_eng
                pending_oproj.pop(0)(**kw)

            def make_oproj(ic, dt):
                def emit(dma_eng=nc.sync, copy_eng=None):
                    # full-slot allocation (only bank 0 used): po shares the
                    # score tiles' tag-a rotation, and matching the slot size
                    # keeps the dependency tracking whole-slot-exact
                    po = psum.tile([P, 2 * IT], F32, tag="a", bufs=3,
                                   name="po")
                    for ec in range(CDIM // P):
                        nc.tensor.matmul(
                            po[:, 0:IT],
                            lhsT=attnTb[:, ec, ic * P:(ic + 1) * P],
                            rhs=wo_sb[:, ec, dt * IT:(dt + 1) * IT],
                            start=(ec == 0), stop=(ec == CDIM // P - 1))
                    ob = opool.tile([P, IT], BF16, tag="ob")
                    # 3:1 scalar:vector — vector carries the rescale chain
                    if copy_eng is nc.vector or (
                            copy_eng is None and oproj_ct[0] % 4 == 3):
                        nc.vector.tensor_copy(ob[:], po[:, 0:IT])
                    else:
                        nc.scalar.copy(ob[:], po[:, 0:IT])
                    oproj_ct[0] += 1
                    dma_eng.dma_start(
                        outp[ic * P:(ic + 1) * P, dt * IT:(dt + 1) * IT], ob[:])
                return emit

            def emit_exp(blk, j, sc, Pt):
                mode = exp_mode(blk, j)
                if mode == 'S':
                    nc.scalar.activation(
                        Pt[:], sc[:], mybir.ActivationFunctionType.Exp,
                        scale=SCALE)
                elif mode == 'F':
                    nc.vector.tensor_scalar(
                        Pt[:].bitcast(I16), sc[:], EXPF_A, EXPF_B,
                        mybir.AluOpType.mult, mybir.AluOpType.add)
                else:  # 'A'
                    sx = sxpool.tile([P, 2, IG], I32, tag="sx")
                    nc.vector.tensor_scalar(
                        sx[:], sc[:], EXPA_A, EXPA_B,
                        mybir.AluOpType.mult, mybir.AluOpType.add)
                    nc.vector._custom_dve(
                        EXP_MANCOR, out=Pt[:], in0=sx[:].bitcast(F32),
                        in1=c2t[:], s0=MASK_F, s1=EXP_C0, imm2=EXP_C1)

            blk = 0
            for ig in range(NIG):
                i0 = ig * IG
                for pp in range(2):
                    hA, hB = 2 * pp, 2 * pp + 1
                    pvs = [psum.tile([P, IG], F32, tag="b", bufs=2,
                                     name=f"pv{x}") for x in range(2)]
                    Pts = {}

                    def pv_mms(j):
                        for x, h in enumerate((hA, hB)):
                            nc.tensor.matmul(
                                pvs[x][:],
                                lhsT=vh[:, j, h, :],
                                rhs=Pts[j][:, x, :],
                                start=(j == 0), stop=(j == NJC - 1))
                        del Pts[j]

                    for j in range(NJC):
                        sc = psum.tile([P, 2, IG], F32, tag="a", bufs=3,
                                       name="sc")
                        # the two heads' K=64 score matmuls go to PE row
                        # groups 0 and 64 (derived from lhsT base_partition)
                        # and execute concurrently in the array
                        nc.tensor.matmul(
                            sc[:, 0, :],
                            lhsT=khT[0:HD, pp, j * P:(j + 1) * P],
                            rhs=qhT[0:HD, pp, i0:i0 + IG],
                            start=True, stop=True)
                        nc.tensor.matmul(
                            sc[:, 1, :],
                            lhsT=khT[HD:P, pp, j * P:(j + 1) * P],
                            rhs=qhT[HD:P, pp, i0:i0 + IG],
                            start=True, stop=True)
                        Pt = ptpool.tile([P, 2, IG], BF16, tag="pt")
                        emit_exp(blk, j, sc, Pt)
                        Pts[j] = Pt
                        if j >= 2:
                            pv_mms(j - 2)
                        # emissions sit late in the block: the previous ig's
                        # rescale chain needs time to land in attnTb, and an
                        # earlier chunk would stall the PE FIFO waiting on
                        # it.  blocks 6-7 skip these so 6 reserved chunks
                        # remain to carry the PE through the final chain
                        if j in (11, 13, 15) and 2 <= blk < 6:
                            emit_oproj_chunk()
                    pv_mms(NJC - 2)
                    pv_mms(NJC - 1)

                    # block tail: 1/d broadcast comes straight from the PSUM
                    # ones-half via a cross-base copy (no DRAM hops): rd at
                    # partitions 0-63 <- pv partitions 64-127 (64 identical
                    # denominator rows), then reciprocal + rescale multiply
                    rd = rpool.tile([HD, 2, IG], F32, tag="rd")
                    nc.vector.tensor_copy(rd[:, 0, :], pvs[0][HD:P, :])
                    nc.vector.tensor_copy(rd[:, 1, :], pvs[1][HD:P, :])
                    rrec = rpool.tile([HD, 2, IG], F32, tag="rrec")
                    nc.vector.reciprocal_approx_fast(out=rrec[:], in_=rd[:])
                    st = stpool.tile([HD, 2, IG], F32, tag="st")
                    nc.scalar.copy(st[:, 0, :], pvs[0][0:HD, :])
                    nc.vector.tensor_copy(st[:, 1, :], pvs[1][0:HD, :])
                    if dbg:
                        nc.scalar.dma_start(
                            d_den[3 * blk:3 * blk + 1, :],
                            rd[0:1].rearrange("p a b -> p (a b)"))
                        nc.scalar.dma_start(
                            d_den[3 * blk + 1:3 * blk + 2, :],
                            rrec[0:1].rearrange("p a b -> p (a b)"))
                        nc.scalar.dma_start(
                            d_den[3 * blk + 2:3 * blk + 3, :],
                            st[0:1].rearrange("p a b -> p (a b)"))
                    # boundary batch: out-proj chunks slot into the PE
                    # bubble while this block's chain latency plays out.
                    # block 7 drains all 6 reserved ig2 chunks here, BEFORE
                    # its attnTb write (chunks emitted after it would falsely
                    # wait on it — dependency tracking is coarse)
                    if blk == 7:
                        for x in range(6):
                            emit_oproj_chunk(
                                dma_eng=nc.scalar if x % 2 else nc.sync,
                                copy_eng=nc.scalar)
                    elif blk == 6:
                        emit_oproj_chunk()
                        emit_oproj_chunk()
                    elif blk >= 2:
                        emit_oproj_chunk()
                    stb = stpool.tile([HD, 2 * IG], BF16, tag="stb")
                    mul_eng = nc.vector if blk == 7 else nc.gpsimd
                    mul_eng.tensor_mul(
                        stb[:], st.rearrange("p a b -> p (a b)"),
                        rrec.rearrange("p a b -> p (a b)"))
                    nc.sync.dma_start(attnTb[0:HD, pp, i0:i0 + IG],
                                      stb[:, 0:IG])
                    nc.sync.dma_start(attnTb[HD:P, pp, i0:i0 + IG],
                                      stb[:, IG:2 * IG])
                    if pp == 1:
                        for icl in range(IG // P):
                            for dt in range(2):
                                pending_oproj.append(
                                    make_oproj(ig * (IG // P) + icl, dt))
                    blk += 1
            fin = 0
            while pending_oproj:
                emit_oproj_chunk(dma_eng=nc.scalar if fin % 2 else nc.sync)
                fin += 1
            if dbg:
                nc.sync.dma_start(d_att, attnTb[:])

    nc.compile()
    return nc


_NC_CACHE = {}


def _get_nc(QL, KL):
    key = (QL, KL)
    if key not in _NC_CACHE:
        _NC_CACHE[key] = build_bass(QL, KL)
    return _NC_CACHE[key]


def _pmajor(wT):
    """[(o p), d] -> [p, o, d]: partition-major weight layout so the device
    DMA is one contiguous block per partition."""
    op, dd = wT.shape
    return np.ascontiguousarray(
        wT.reshape(op // P, P, dd).transpose(1, 0, 2))


def make_in_maps(q, k, v, Wq, Wk, Wv, Wo):
    """Per-core input maps (bf16, weights pre-transposed + partition-major)."""
    bf = ml_dtypes.bfloat16
    q, k, v = (np.asarray(x, np.float32) for x in (q, k, v))
    WqT = np.asarray(Wq, np.float32).T.astype(bf)
    WkT = np.asarray(Wk, np.float32).T.astype(bf)
    WvT = np.asarray(Wv, np.float32).T.astype(bf)
    WoT = np.asarray(Wo, np.float32).T.astype(bf)
    qb = [np.ascontiguousarray(q[b].T.astype(bf)) for b in range(B)]
    kb = [np.ascontiguousarray(k[b].T.astype(bf)) for b in range(B)]
    vb = [np.ascontiguousarray(v[b].T.astype(bf)) for b in range(B)]
    in_maps = []
    for c in range(NCORES):
        b, hs = c // 4, c % 4
        sl = slice(hs * CDIM, (hs + 1) * CDIM)
        in_maps.append({
            "qbT": qb[b], "kbT": kb[b], "vbT": vb[b],
            "wqT": _pmajor(WqT[:, sl]),
            "wkT": _pmajor(WkT[:, sl]),
            "wvT": _pmajor(WvT[:, sl]),
            "woT": _pmajor(WoT[sl, :]),
        })
    return in_maps


def kernel(q, k, v, Wq, Wk, Wv, Wo, bo, _trace=False):
    q = np.asarray(q, np.float32)
    QL, KL = q.shape[1], np.asarray(k).shape[1]
    nc = _get_nc(QL, KL)
    in_maps = make_in_maps(q, k, v, Wq, Wk, Wv, Wo)
    res = run_bass_kernel_spmd(nc, in_maps, core_ids=list(range(NCORES)),
                               trace=_trace)
    bo = np.asarray(bo, np.float32)
    out = np.empty((B, QL, DIM), np.float32)
    for b in range(B):
        acc = res.results[4 * b]["outp"].astype(np.float32)
        for c in range(4 * b + 1, 4 * b + 4):
            acc += res.results[c]["outp"].astype(np.float32)
        out[b] = acc + bo
    if _trace:
        kernel._last_results = res
    return out
